# revision 6
# baseline (speedup 1.0000x reference)
"""Multi-head attention (B=8, N=1024, C=768, H=12) on 8 TRN2 NeuronCores.

Data-parallel: one batch element per core. Feature-major on-chip layout
(no transposes). All matmul operands arrive pre-cast to bf16 from the host,
so no on-chip input casting is needed.

Per core, a software-pipelined slot schedule paced by the ScalarE exp
stream. For each head pair p (6 pairs), 16 slots of (nch, mt):

  slot: 2 score MMs  S^T = K_h @ Q_h^T   (K=64 row-halves 0:64 / 64:128 of
        the PE array -> the two MMs execute concurrently, ~108 ns/MM)
        -> one [128,1024] PSUM tile -> ScalarE exp -> bf16 P^T tile
  interleaved into the same slots:
        AV MMs of pair p-1:  [V_h | 1]^T @ P^T -> pav[65, 512] (Z in row 64)
        QK MMs of pair p+1 and V MMs (work-pool PSUM, DVE evacuation)
  softmax-normalize: DVE reciprocal of Z row, GpSimd partition-broadcast,
        DVE multiply -> attnT (bf16)
  tail: y^T = w_proj @ attn^T + b (PSUM accum, ScalarE bias-identity, DMA).

PSUM bank budget (8 banks): scores 2x[128,1024] (4) + pav 2x[65,512] (2)
+ work 2x[128,512] (2).
"""

import sys

if "/opt/trn_rl_repo" not in sys.path:
    sys.path.insert(0, "/opt/trn_rl_repo")

import numpy as np

import concourse.bass as bass  # noqa: F401
import concourse.mybir as mybir
import concourse.tile as tile
from concourse import bacc
from concourse.bass_utils import run_bass_kernel_spmd

F32 = mybir.dt.float32
F32R = mybir.dt.float32r
BF16 = mybir.dt.bfloat16
AF = mybir.ActivationFunctionType

B, N, C = 8, 1024, 768
H, D = 12, 64
SCALE = D ** -0.5
KT = C // 128    # 6 contraction tiles
NT = N // 128    # 8 token tiles
PAIRS = H // 2   # 6 head pairs

_CACHE = {}


def build():
    nc = bacc.Bacc("TRN2", target_bir_lowering=False, debug=False, num_devices=8)

    xT_d = nc.dram_tensor("xT", [C, N], BF16, kind="ExternalInput")
    wqk_d = nc.dram_tensor("w_qkT", [C, 2 * C], BF16, kind="ExternalInput")
    wv_d = nc.dram_tensor("w_vT", [C, C], BF16, kind="ExternalInput")
    wp_d = nc.dram_tensor("w_pT", [C, C], BF16, kind="ExternalInput")
    b_d = nc.dram_tensor("b_p", [C, 1], F32, kind="ExternalInput")
    out_d = nc.dram_tensor("out", [C, N], F32, kind="ExternalOutput")

    with tile.TileContext(nc) as tc:
        _body(nc, tc, xT_d, wqk_d, wv_d, wp_d, b_d, out_d)
    nc.compile()
    return nc


def _body(nc, tc, xT_d, wqk_d, wv_d, wp_d, b_d, out_d):
    from contextlib import ExitStack

    with ExitStack() as ctx:
        ctx.enter_context(
            nc.allow_low_precision(reason="bf16 matmul operands; accum stays f32")
        )
        const = ctx.enter_context(tc.tile_pool(name="const", bufs=1))
        w_pool = ctx.enter_context(tc.tile_pool(name="w", bufs=1))
        qk_pool = ctx.enter_context(tc.tile_pool(name="qk", bufs=1))
        v_pool = ctx.enter_context(tc.tile_pool(name="v", bufs=1))
        attn_pool = ctx.enter_context(tc.tile_pool(name="attn", bufs=1))
        pt_pool = ctx.enter_context(tc.tile_pool(name="pt", bufs=32))
        nrm_pool = ctx.enter_context(tc.tile_pool(name="nrm", bufs=2))
        y_pool = ctx.enter_context(tc.tile_pool(name="y", bufs=3))
        ps_s = ctx.enter_context(tc.tile_pool(name="ps_s", bufs=2, space="PSUM"))
        ps_av = ctx.enter_context(tc.tile_pool(name="ps_av", bufs=2, space="PSUM"))
        ps_w = ctx.enter_context(tc.tile_pool(name="ps_w", bufs=2, space="PSUM"))

        # ---------------- static SBUF tensors + input DMA ----------------
        xT = [w_pool.tile([128, N], BF16, tag=f"x{i}", name=f"x{i}") for i in range(KT)]
        wqk = [w_pool.tile([128, 2 * C], BF16, tag=f"wqk{i}", name=f"wqk{i}") for i in range(KT)]
        wv = [w_pool.tile([128, C], BF16, tag=f"wv{i}", name=f"wv{i}") for i in range(KT)]
        wp = [w_pool.tile([128, C], BF16, tag=f"wp{i}", name=f"wp{i}") for i in range(KT)]
        b_sb = const.tile([128, KT], F32)
        for kt in range(KT):
            ksl = slice(kt * 128, (kt + 1) * 128)
            nc.sync.dma_start(out=xT[kt], in_=xT_d.ap()[ksl, :])
            nc.sync.dma_start(out=wqk[kt], in_=wqk_d.ap()[ksl, :])
            nc.sync.dma_start(out=wv[kt], in_=wv_d.ap()[ksl, :])
            nc.sync.dma_start(out=wp[kt], in_=wp_d.ap()[ksl, :])
            nc.sync.dma_start(out=b_sb[:, kt : kt + 1], in_=b_d.ap()[ksl, :])

        ones12 = const.tile([128, H], BF16)
        nc.vector.memset(ones12, 1.0)
        ones_raw = const.tile([1, 64], F32)
        nc.vector.memset(ones_raw, 1.0)
        ones_col = const.tile([1, 64], F32R)
        nc.vector.tensor_copy(ones_col, ones_raw)

        qkT = [qk_pool.tile([128, N], BF16, tag=f"qkT{i}", name=f"qkT{i}") for i in range(2 * KT)]
        v_sb = [v_pool.tile([128, H, 65], BF16, tag=f"v{i}", name=f"v{i}") for i in range(NT)]
        attnT = [attn_pool.tile([128, N], BF16, tag=f"at{i}", name=f"at{i}") for i in range(KT)]

        # ---------------- emission helpers ----------------
        def qk_unit(ot, nch):
            """QK output tile ot (0-5 q, 6-11 k), token chunk nch. 7 steps."""
            nsl = slice(nch * 512, (nch + 1) * 512)
            osl = slice(ot * 128, (ot + 1) * 128)
            w = [None]

            def mk(kt):
                def f():
                    if kt == 0:
                        w[0] = ps_w.tile([128, 512], F32, tag="w", name=f"qkw{ot}_{nch}")
                    nc.tensor.matmul(
                        w[0], lhsT=wqk[kt][:, osl], rhs=xT[kt][:, nsl],
                        start=(kt == 0), stop=(kt == KT - 1),
                    )
                return f

            steps = [mk(kt) for kt in range(KT)]
            steps.append(lambda: nc.vector.tensor_copy(qkT[ot][:, nsl], w[0]))
            return steps

        def v_unit(mt, o0, ow):
            """V for token tile mt, feature chunk [o0, o0+ow). 7 steps."""
            msl = slice(mt * 128, (mt + 1) * 128)
            nh = ow // 64
            w = [None]

            def mk(kt):
                def f():
                    if kt == 0:
                        w[0] = ps_w.tile([128, 512], F32, tag="w", name=f"vw{mt}_{o0}")
                    nc.tensor.matmul(
                        w[0][:, :ow], lhsT=xT[kt][:, msl], rhs=wv[kt][:, o0 : o0 + ow],
                        start=(kt == 0), stop=(kt == KT - 1),
                    )
                return f

            steps = [mk(kt) for kt in range(KT)]

            def fin():
                dst = v_sb[mt][:, o0 // 64 : o0 // 64 + nh, 0:64]
                vsrc = w[0][:, :ow].rearrange("p (h e) -> p h e", e=64)
                nc.vector.tensor_copy(dst, vsrc)

            steps.append(fin)
            return steps

        def vones_unit(mt):
            return [lambda: nc.vector.tensor_copy(v_sb[mt][:, :, 64:65], ones12.unsqueeze(-1))]

        pt_store = {}

        def scores_exp(p, s):
            nch, mt = s // 8, s % 8
            nsl = slice(nch * 512, (nch + 1) * 512)
            msl = slice(mt * 128, (mt + 1) * 128)
            q_t, k_t = qkT[p], qkT[6 + p]
            pss = ps_s.tile([128, 1024], F32, tag="s", name=f"pss{p}_{s}")
            nc.tensor.matmul(pss[:, 0:512], lhsT=k_t[0:64, msl], rhs=q_t[0:64, nsl],
                             start=True, stop=True)
            nc.tensor.matmul(pss[:, 512:1024], lhsT=k_t[64:128, msl], rhs=q_t[64:128, nsl],
                             start=True, stop=True)
            pt = pt_pool.tile([128, 1024], BF16, tag="pt", name=f"pt{p}_{s}")
            nc.scalar.activation(pt, pss, AF.Exp)
            pt_store[(p, s)] = pt

        pav_store = {}

        def av_slot(q, s):
            """Two AV matmuls for pair q at slot s; norm after mt 7."""
            nch, mt = s // 8, s % 8
            nsl = slice(nch * 512, (nch + 1) * 512)
            pt = pt_store[(q, s)]
            for e in range(2):
                h = 2 * q + e
                if mt == 0:
                    pav_store[(q, nch, e)] = ps_av.tile(
                        [65, 512], F32, tag="av", name=f"pav{q}_{nch}_{e}"
                    )
                nc.tensor.matmul(
                    pav_store[(q, nch, e)],
                    lhsT=v_sb[mt][:, h, :],
                    rhs=pt[:, e * 512 : (e + 1) * 512],
                    start=(mt == 0),
                    stop=(mt == NT - 1),
                )
            if mt == NT - 1:
                for e in range(2):
                    norm(q, nch, e, nsl)

        def norm(q, nch, e, nsl):
            import os

            pav = pav_store[(q, nch, e)]
            dst = attnT[q][e * 64 : e * 64 + 64, nsl]
            if os.environ.get("NORM_MODE", "pe") == "gpsimd":
                r = nrm_pool.tile([1, 512], F32, tag="r", name=f"r{q}_{nch}_{e}")
                nc.vector.reciprocal_approx_fast(out=r, in_=pav[64:65, :])
                rb = nrm_pool.tile([64, 512], F32, tag="rb", name=f"rb{q}_{nch}_{e}")
                nc.gpsimd.partition_broadcast(rb, r, channels=64)
                nc.vector.tensor_mul(dst, pav[0:64, :], rb)
            else:
                # Z row -> f32r SBUF, broadcast via K=1 matmul on PE, fast
                # reciprocal on 64 lanes, one multiply (baseline-proven).
                z_sb = nrm_pool.tile([1, 512], F32R, tag="z", name=f"z{q}_{nch}_{e}")
                nc.vector.tensor_copy(z_sb, pav[64:65, :])
                zb = ps_w.tile([64, 512], F32, tag="w", name=f"zb{q}_{nch}_{e}")
                nc.tensor.matmul(zb, lhsT=ones_col, rhs=z_sb, start=True, stop=True)
                av_r = nrm_pool.tile([64, 512], F32R, tag="avr", name=f"avr{q}_{nch}_{e}")
                nc.vector.tensor_copy(av_r, pav[0:64, :])
                zr64 = nrm_pool.tile([64, 512], F32, tag="zr", name=f"zr{q}_{nch}_{e}")
                nc.vector.reciprocal_approx_fast(out=zr64, in_=zb)
                nc.vector.tensor_mul(dst, zr64, av_r)

        def proj_unit(ot, nch):
            nsl = slice(nch * 512, (nch + 1) * 512)
            osl = slice(ot * 128, (ot + 1) * 128)
            w = ps_w.tile([128, 512], F32, tag="w", name=f"pw{ot}_{nch}")
            for kt in range(KT):
                nc.tensor.matmul(
                    w, lhsT=wp[kt][:, osl], rhs=attnT[kt][:, nsl],
                    start=(kt == 0), stop=(kt == KT - 1),
                )
            y = y_pool.tile([128, 512], F32, tag="y")
            nc.scalar.activation(y, w, AF.Identity, bias=b_sb[:, ot : ot + 1])
            nc.sync.dma_start(out=out_d.ap()[osl, nsl], in_=y)

        # ---------------- schedule ----------------
        # pair -1 prelude: QK for pair 0 (both token chunks), emitted densely.
        for step in [st for nch in range(2) for u in (qk_unit(0, nch), qk_unit(6, nch)) for st in u]:
            step()

        # background work queues per pair window:
        #  pair 0: QK of pair 1 + all V-512 units (heads 0-7) + ones cols
        #  pairs 1-3: QK of pair p+1 + a third of the V-256 units
        #  pair 4: QK of pair 5
        bg = {p: [] for p in range(PAIRS)}
        for p in range(PAIRS - 1):
            r = p + 1
            for nch in range(2):
                bg[p].extend(qk_unit(r, nch))
                bg[p].extend(qk_unit(6 + r, nch))
        for mt in range(NT):
            bg[0].extend(vones_unit(mt))
            bg[0].extend(v_unit(mt, 0, 512))
        for i, mt in enumerate(range(NT)):
            bg[1 + i * 3 // NT].extend(v_unit(mt, 512, 256))

        for p in range(PAIRS):
            items = bg[p]
            L = len(items)
            for s in range(16):
                scores_exp(p, s)
                if p >= 1:
                    av_slot(p - 1, s)
                for i in range(L * s // 16, L * (s + 1) // 16):
                    items[i]()

        for s in range(16):
            av_slot(PAIRS - 1, s)

        for ot in range(KT):
            for nch in range(2):
                proj_unit(ot, nch)


def _get_nc():
    if "nc" not in _CACHE:
        _CACHE["nc"] = build()
    return _CACHE["nc"]


def kernel(x, w_qkv, w_proj, b_proj, _trace=False):
    import ml_dtypes

    bf16 = ml_dtypes.bfloat16
    x = np.asarray(x, dtype=np.float32)
    w_qkv = np.asarray(w_qkv, dtype=np.float32)
    w_proj = np.asarray(w_proj, dtype=np.float32)
    b_proj = np.asarray(b_proj, dtype=np.float32)

    wq = w_qkv[0:C] * np.float32(SCALE)
    wk = w_qkv[C : 2 * C]
    wv = w_qkv[2 * C : 3 * C]
    w_qkT = np.ascontiguousarray(np.concatenate([wq, wk], axis=0).T).astype(bf16)
    w_vT = np.ascontiguousarray(wv.T).astype(bf16)
    w_pT = np.ascontiguousarray(w_proj.T).astype(bf16)
    b_p = np.ascontiguousarray(b_proj.reshape(C, 1))

    in_maps = []
    for i in range(B):
        in_maps.append(
            {
                "xT": np.ascontiguousarray(x[i].T).astype(bf16),
                "w_qkT": w_qkT,
                "w_vT": w_vT,
                "w_pT": w_pT,
                "b_p": b_p,
            }
        )

    nc = _get_nc()
    res = run_bass_kernel_spmd(nc, in_maps, core_ids=list(range(B)), trace=_trace)
    _CACHE["last_result"] = res

    out = np.empty((B, N, C), dtype=np.float32)
    for i in range(B):
        out[i] = res.results[i]["out"].T
    return out
